# revision 14
# baseline (speedup 1.0000x reference)
"""Bahdanau cross-attention kernel for Trainium2 (8 NeuronCores, SPMD).

reference:
    S = dec @ enc^T            [B, Tq, Tk]
    P = softmax(S, axis=-1)    (output 2)
    O = P @ enc                [B, Tq, H]  (output 1)

Sharding: data-parallel over batch — B=16 split as 2 batches per core.
No collectives needed; each core computes its batches independently.

Per-core algorithm (per batch):
  - load enc natural, round to f32r (matmul2 rhs), and build encT
    (h on partitions) via PE transposes (matmul1 rhs).
  - per q-tile of 128 rows:
      decT via PE transposes (matmul1 stationary);
      matmul1 in 4 k-blocks of 512 accumulated over 8 h-chunks (f32r,
      full-rate); per-block rowmax + exp(S - m_b) with accumulated
      row-sums (frees the PSUM bank right away, flash-style);
      block rescale factors fix_b = e^{m_b - m}/sum folded into a single
      per-block in-place normalize -> P (f32r);
      P -> DRAM; PE-transpose P -> PT; matmul2 (PT stationary, enc_r
      moving, f32r) -> O -> DRAM.
"""

from contextlib import ExitStack

import numpy as np

import concourse.bass as bass
import concourse.tile as tile
import concourse.mybir as mybir
from concourse import bacc
from concourse.bass_utils import run_bass_kernel_spmd
from concourse.masks import make_identity

F32 = mybir.dt.float32
F32R = mybir.dt.float32r
BF16 = mybir.dt.bfloat16
AF = mybir.ActivationFunctionType
AX = mybir.AxisListType
ALU = mybir.AluOpType

N_CORES = 8


def build_attention(B_pc: int, Tq: int, Tk: int, H: int):
    assert Tq % 128 == 0 and Tk % 512 == 0 and H % 128 == 0
    kt = Tk // 128  # k tiles (PT / mm2 contraction chunks)
    hc = H // 128  # h chunks (mm1 contraction chunks)
    nkb = Tk // 512  # S blocks (512-wide)
    nqt = Tq // 128  # q tiles
    hb = min(H, 512)  # mm2 moving-block width
    nhb = H // hb

    nc = bacc.Bacc("TRN2", target_bir_lowering=False, debug=False, num_devices=N_CORES)
    enc_d = nc.dram_tensor("encoder_outputs", [B_pc, Tk, H], F32, kind="ExternalInput")
    dec_d = nc.dram_tensor("decoder_hidden", [B_pc, Tq, H], F32, kind="ExternalInput")
    o_d = nc.dram_tensor("attention_output", [B_pc, Tq, H], F32, kind="ExternalOutput")
    p_d = nc.dram_tensor(
        "attention_distribution", [B_pc, Tq, Tk], F32, kind="ExternalOutput"
    )

    with tile.TileContext(nc) as tc, ExitStack() as ctx:
        singles = ctx.enter_context(tc.tile_pool(name="singles", bufs=1))
        encp = ctx.enter_context(tc.tile_pool(name="encp", bufs=1))
        staging = ctx.enter_context(tc.tile_pool(name="staging", bufs=3))
        decp = ctx.enter_context(tc.tile_pool(name="decp", bufs=3))
        expp = ctx.enter_context(tc.tile_pool(name="expp", bufs=2))
        pbp = ctx.enter_context(tc.tile_pool(name="pbp", bufs=2))
        ptp = ctx.enter_context(tc.tile_pool(name="ptp", bufs=2))
        op = ctx.enter_context(tc.tile_pool(name="op", bufs=2))
        stats = ctx.enter_context(tc.tile_pool(name="stats", bufs=4))
        ps_s = ctx.enter_context(tc.tile_pool(name="ps_s", bufs=4, space="PSUM"))
        ps_t = ctx.enter_context(tc.tile_pool(name="ps_t", bufs=2, space="PSUM"))
        ps_o = ctx.enter_context(tc.tile_pool(name="ps_o", bufs=1, space="PSUM"))

        ident = singles.tile([128, 128], F32)
        make_identity(nc, ident[:])
        tc.strict_bb_all_engine_barrier()

        for b in range(B_pc):
            # ---- per-batch enc setup: enc_b (k-major, bf16, mm2 rhs) +
            #      encT (h-major, f32r, mm1 rhs)
            encb = encp.tile([128, kt, H], BF16, tag="encb")
            encT = encp.tile([128, hc, Tk], F32R, tag="encT")
            for t in range(kt):
                st = staging.tile([128, H], F32)
                nc.sync.dma_start(st[:], enc_d.ap()[b, t * 128 : (t + 1) * 128, :])
                nc.vector.tensor_copy(encb[:, t, :], st[:])  # cast f32 -> bf16
                for h in range(hc):
                    ptile = ps_t.tile([128, 4, 128], F32, tag="pst")
                    nc.tensor.transpose(
                        ptile[:, h % 4, :], st[:, h * 128 : (h + 1) * 128], ident[:]
                    )
                    nc.scalar.copy(
                        encT[:, h, t * 128 : (t + 1) * 128], ptile[:, h % 4, :]
                    )

            # ---- q-tile loop, software-pipelined:
            # iteration q emits [dec-prep](q+1), [mm1/softmax/PTb](q),
            # then [mm2/out](q-1)
            def dec_prep(q):
                dn = staging.tile([128, H], F32, tag="dec_nat", name=f"dn{q}")
                nc.sync.dma_start(dn[:], dec_d.ap()[b, q * 128 : (q + 1) * 128, :])
                decT = decp.tile([128, hc, 128], F32R, tag="decT", name=f"decT{q}")
                for h in range(hc):
                    ptile = ps_t.tile([128, 4, 128], F32, tag="pst", name=f"pt{q}_{h}")
                    nc.tensor.transpose(
                        ptile[:, h % 4, :], dn[:, h * 128 : (h + 1) * 128], ident[:]
                    )
                    nc.vector.tensor_copy(decT[:, h, :], ptile[:, h % 4, :])
                return decT

            decTs = {0: dec_prep(0)}
            carry = None
            for q in range(nqt + 1):
                if q + 1 < nqt:
                    decTs[q + 1] = dec_prep(q + 1)
                if q < nqt:
                    decT = decTs.pop(q)
                    expS = expp.tile([128, Tk], F32R)
                    rowmaxneg = stats.tile([128, nkb], F32, tag="rmn")
                    sums = stats.tile([128, nkb], F32, tag="sums")
                    # h-outer so 4 consecutive matmuls share the same
                    # stationary operand (saves weight reloads)
                    Sblk = [
                        ps_s.tile([128, 512], F32, tag="S", name=f"S{kb}")
                        for kb in range(nkb)
                    ]
                    for h in range(hc):
                        for kb in range(nkb):
                            nc.tensor.matmul(
                                Sblk[kb][:],
                                decT[:, h, :],
                                encT[:, h, kb * 512 : (kb + 1) * 512],
                                start=(h == 0),
                                stop=(h == hc - 1),
                            )
                    for kb in range(nkb):
                        nc.vector.reduce_max(
                            out=rowmaxneg[:, kb : kb + 1],
                            in_=Sblk[kb][:],
                            axis=AX.X,
                            negate=True,
                        )
                        nc.scalar.activation(
                            expS[:, kb * 512 : (kb + 1) * 512],
                            Sblk[kb][:],
                            AF.Exp,
                            bias=rowmaxneg[:, kb : kb + 1],
                            scale=1.0,
                            accum_out=sums[:, kb : kb + 1],
                        )
                    # block-combine stats:
                    #   m_b = -rowmaxneg_b ; m = max_b m_b
                    #   fix_b = e^{m_b - m} / sum_b' (sums_b' e^{m_b' - m})
                    minneg = stats.tile([128, 1], F32, tag="minneg")
                    nc.vector.tensor_reduce(
                        out=minneg[:], in_=rowmaxneg[:], axis=AX.X, op=ALU.min
                    )
                    eb = stats.tile([128, nkb], F32, tag="eb")
                    dd = stats.tile([128, nkb], F32, tag="dd")
                    nc.vector.tensor_scalar(
                        out=dd[:],
                        in0=rowmaxneg[:],
                        scalar1=minneg[:],
                        scalar2=None,
                        op0=ALU.subtract,
                    )
                    nc.scalar.activation(eb[:], dd[:], AF.Exp, scale=-1.0)
                    wb = stats.tile([128, nkb], F32, tag="wb")
                    nc.vector.tensor_mul(wb[:], sums[:], eb[:])
                    ssum = stats.tile([128, 1], F32, tag="ssum")
                    nc.vector.reduce_sum(out=ssum[:], in_=wb[:], axis=AX.X)
                    rec = stats.tile([128, 1], F32, tag="rec")
                    nc.vector.reciprocal(rec[:], ssum[:])
                    fix = stats.tile([128, nkb], F32, tag="fix")
                    nc.vector.tensor_scalar_mul(fix[:], eb[:], rec[:])
                    # in-place normalize: expS_b *= fix_b  -> P (f32r)
                    for kb in range(nkb):
                        nc.vector.tensor_scalar_mul(
                            expS[:, kb * 512 : (kb + 1) * 512],
                            expS[:, kb * 512 : (kb + 1) * 512],
                            fix[:, kb : kb + 1],
                        )
                    nc.sync.dma_start(
                        p_d.ap()[b, q * 128 : (q + 1) * 128, :],
                        expS[:].bitcast(F32),
                    )
                    # bf16 copy of P, then per-tile DMA xbar transposes -> PTb
                    Pb = pbp.tile([128, Tk], BF16)
                    nc.vector.tensor_copy(Pb[:], expS[:])
                    PTb = ptp.tile([128, kt, 128], BF16)
                    for t in range(kt):
                        nc.sync.dma_start(
                            PTb[:, t, :],
                            Pb[:, t * 128 : (t + 1) * 128],
                            transpose=True,
                        )
                    this = PTb

                if carry is not None:
                    PTb_p, qp = carry
                    O = ps_o.tile([128, H], F32)
                    # t-outer so both h-halves share the stationary PTb[t]
                    for t in range(kt):
                        for nb in range(nhb):
                            nc.tensor.matmul(
                                O[:, nb * hb : (nb + 1) * hb],
                                PTb_p[:, t, :],
                                encb[:, t, nb * hb : (nb + 1) * hb],
                                start=(t == 0),
                                stop=(t == kt - 1),
                            )
                    Os = op.tile([128, H], F32)
                    nc.scalar.copy(Os[:], O[:])
                    nc.sync.dma_start(o_d.ap()[b, qp * 128 : (qp + 1) * 128, :], Os[:])

                carry = (this, q) if q < nqt else None

    nc.compile()
    return nc


def kernel(encoder_outputs: np.ndarray, decoder_hidden: np.ndarray):
    encoder_outputs = np.ascontiguousarray(encoder_outputs, dtype=np.float32)
    decoder_hidden = np.ascontiguousarray(decoder_hidden, dtype=np.float32)
    B, Tk, H = encoder_outputs.shape
    Tq = decoder_hidden.shape[1]
    assert B % N_CORES == 0
    B_pc = B // N_CORES

    nc = build_attention(B_pc, Tq, Tk, H)
    in_maps = [
        {
            "encoder_outputs": encoder_outputs[i * B_pc : (i + 1) * B_pc],
            "decoder_hidden": decoder_hidden[i * B_pc : (i + 1) * B_pc],
        }
        for i in range(N_CORES)
    ]
    res = run_bass_kernel_spmd(nc, in_maps, core_ids=list(range(N_CORES)))
    ao = np.concatenate(
        [res.results[i]["attention_output"] for i in range(N_CORES)], axis=0
    )
    ad = np.concatenate(
        [res.results[i]["attention_distribution"] for i in range(N_CORES)], axis=0
    )
    return ao, ad


# revision 17
# speedup vs baseline: 1.6084x; 1.6084x over previous
"""Bahdanau cross-attention kernel for Trainium2 (8 NeuronCores, SPMD).

reference:
    S = dec @ enc^T            [B, Tq, Tk]
    P = softmax(S, axis=-1)    (output 2)
    O = P @ enc                [B, Tq, H]  (output 1)

Sharding: data-parallel over batch — B=16 split as 2 batches per core.
No collectives needed; each core computes its batches independently.

Per-core algorithm (per batch):
  - load enc natural, round to f32r (matmul2 rhs), and build encT
    (h on partitions) via PE transposes (matmul1 rhs).
  - per q-tile of 128 rows:
      decT via PE transposes (matmul1 stationary);
      matmul1 in 4 k-blocks of 512 accumulated over 8 h-chunks (f32r,
      full-rate); per-block rowmax + exp(S - m_b) with accumulated
      row-sums (frees the PSUM bank right away, flash-style);
      block rescale factors fix_b = e^{m_b - m}/sum folded into a single
      per-block in-place normalize -> P (f32r);
      P -> DRAM; PE-transpose P -> PT; matmul2 (PT stationary, enc_r
      moving, f32r) -> O -> DRAM.
"""

from contextlib import ExitStack

import numpy as np

import concourse.bass as bass
import concourse.tile as tile
import concourse.mybir as mybir
from concourse import bacc
from concourse.bass_utils import run_bass_kernel_spmd
from concourse.masks import make_identity

F32 = mybir.dt.float32
F32R = mybir.dt.float32r
BF16 = mybir.dt.bfloat16
AF = mybir.ActivationFunctionType
AX = mybir.AxisListType
ALU = mybir.AluOpType

N_CORES = 8


def build_attention(B_pc: int, Tq: int, Tk: int, H: int):
    assert Tq % 128 == 0 and Tk % 512 == 0 and H % 128 == 0
    kt = Tk // 128  # k tiles (PT / mm2 contraction chunks)
    hc = H // 128  # h chunks (mm1 contraction chunks)
    nkb = Tk // 512  # S blocks (512-wide)
    nqt = Tq // 128  # q tiles
    hb = min(H, 512)  # mm2 moving-block width
    nhb = H // hb

    nc = bacc.Bacc("TRN2", target_bir_lowering=False, debug=False, num_devices=N_CORES)
    enc_d = nc.dram_tensor("encoder_outputs", [B_pc, Tk, H], F32, kind="ExternalInput")
    dec_d = nc.dram_tensor("decoder_hidden", [B_pc, Tq, H], F32, kind="ExternalInput")
    o_d = nc.dram_tensor("attention_output", [B_pc, Tq, H], F32, kind="ExternalOutput")
    p_d = nc.dram_tensor(
        "attention_distribution", [B_pc, Tq, Tk], F32, kind="ExternalOutput"
    )

    with tile.TileContext(nc) as tc, ExitStack() as ctx:
        singles = ctx.enter_context(tc.tile_pool(name="singles", bufs=1))
        encp = ctx.enter_context(tc.tile_pool(name="encp", bufs=1))
        staging = ctx.enter_context(tc.tile_pool(name="staging", bufs=3))
        decp = ctx.enter_context(tc.tile_pool(name="decp", bufs=3))
        expp = ctx.enter_context(tc.tile_pool(name="expp", bufs=2))
        pbp = ctx.enter_context(tc.tile_pool(name="pbp", bufs=2))
        ptp = ctx.enter_context(tc.tile_pool(name="ptp", bufs=2))
        op = ctx.enter_context(tc.tile_pool(name="op", bufs=2))
        stats = ctx.enter_context(tc.tile_pool(name="stats", bufs=4))
        ps_s = ctx.enter_context(tc.tile_pool(name="ps_s", bufs=4, space="PSUM"))
        ps_t = ctx.enter_context(tc.tile_pool(name="ps_t", bufs=2, space="PSUM"))
        ps_o = ctx.enter_context(tc.tile_pool(name="ps_o", bufs=1, space="PSUM"))

        ident = singles.tile([128, 128], F32)
        make_identity(nc, ident[:])
        identr = singles.tile([128, 128], F32R)
        nc.vector.tensor_copy(identr[:], ident[:])  # rounds (exact for 0/1)
        tc.strict_bb_all_engine_barrier()

        for b in range(B_pc):
            # ---- per-batch enc setup: enc_b (k-major, bf16, mm2 rhs) +
            #      encT (h-major, f32r, mm1 rhs)
            encb = encp.tile([128, kt, H], BF16, tag="encb")
            encT = encp.tile([128, hc, Tk], F32R, tag="encT")
            for t in range(kt):
                st = staging.tile([128, H], F32)
                nc.sync.dma_start(st[:], enc_d.ap()[b, t * 128 : (t + 1) * 128, :])
                nc.vector.tensor_copy(encb[:, t, :], st[:])  # cast f32 -> bf16
                for h in range(hc):
                    ptile = ps_t.tile([128, 4, 128], F32, tag="pst")
                    nc.tensor.transpose(
                        ptile[:, h % 4, :], st[:, h * 128 : (h + 1) * 128], ident[:]
                    )
                    if h % 2 == 0:
                        nc.scalar.copy(
                            encT[:, h, t * 128 : (t + 1) * 128], ptile[:, h % 4, :]
                        )
                    else:
                        nc.vector.tensor_copy(
                            encT[:, h, t * 128 : (t + 1) * 128], ptile[:, h % 4, :]
                        )

            # ---- q-tile loop, software-pipelined:
            # iteration q emits [dec-prep](q+1), [mm1/softmax/PTb](q),
            # then [mm2/out](q-1)
            def dec_prep(q):
                dn = staging.tile([128, H], F32, tag="dec_nat", name=f"dn{q}")
                nc.sync.dma_start(dn[:], dec_d.ap()[b, q * 128 : (q + 1) * 128, :])
                decT = decp.tile([128, hc, 128], F32R, tag="decT", name=f"decT{q}")
                for h in range(hc):
                    ptile = ps_t.tile([128, 4, 128], F32, tag="pst", name=f"pt{q}_{h}")
                    nc.tensor.transpose(
                        ptile[:, h % 4, :], dn[:, h * 128 : (h + 1) * 128], ident[:]
                    )
                    nc.vector.tensor_copy(decT[:, h, :], ptile[:, h % 4, :])
                return decT

            decTs = {0: dec_prep(0)}
            carry = None
            for q in range(nqt + 1):
                if q + 1 < nqt:
                    decTs[q + 1] = dec_prep(q + 1)
                this = None
                if q < nqt:
                    decT = decTs.pop(q)
                    expS = expp.tile([128, Tk], F32R)
                    rowmaxneg = stats.tile([128, nkb], F32, tag="rmn")
                    sums = stats.tile([128, nkb], F32, tag="sums")
                    # h-outer so 4 consecutive matmuls share the same
                    # stationary operand
                    Sblk = [
                        ps_s.tile([128, 512], F32, tag="S", name=f"S{kb}")
                        for kb in range(nkb)
                    ]
                    for h in range(hc):
                        for kb in range(nkb):
                            nc.tensor.matmul(
                                Sblk[kb][:],
                                decT[:, h, :],
                                encT[:, h, kb * 512 : (kb + 1) * 512],
                                start=(h == 0),
                                stop=(h == hc - 1),
                            )
                    # global row max (negated) -> single exp bias; matmul2
                    # consumes UNNORMALIZED expS so nothing downstream of
                    # exp gates the PE; normalization folds into the O
                    # copy-back (per-partition scale) and the P write-out.
                    for kb in range(nkb):
                        nc.vector.reduce_max(
                            out=rowmaxneg[:, kb : kb + 1],
                            in_=Sblk[kb][:],
                            axis=AX.X,
                            negate=True,
                        )
                    mneg = stats.tile([128, 1], F32, tag="mneg")
                    nc.vector.tensor_reduce(
                        out=mneg[:], in_=rowmaxneg[:], axis=AX.X, op=ALU.min
                    )
                    for kb in range(nkb):
                        nc.scalar.activation(
                            expS[:, kb * 512 : (kb + 1) * 512],
                            Sblk[kb][:],
                            AF.Exp,
                            bias=mneg[:],
                            scale=1.0,
                            accum_out=sums[:, kb : kb + 1],
                        )
                    ssum = stats.tile([128, 1], F32, tag="ssum")
                    nc.vector.reduce_sum(out=ssum[:], in_=sums[:], axis=AX.X)
                    rec = stats.tile([128, 1], F32, tag="rec")
                    nc.vector.reciprocal(rec[:], ssum[:])

                if carry is not None:
                    PTb_p, rec_p, qp = carry
                    O = ps_o.tile([128, H], F32)
                    # t-outer so both h-halves share the stationary PTb[t]
                    for t in range(kt):
                        for nb in range(nhb):
                            nc.tensor.matmul(
                                O[:, nb * hb : (nb + 1) * hb],
                                PTb_p[:, t, :],
                                encb[:, t, nb * hb : (nb + 1) * hb],
                                start=(t == 0),
                                stop=(t == kt - 1),
                            )
                    Os = op.tile([128, H], F32)
                    # copy-back fused with the 1/rowsum normalization
                    nc.scalar.mul(Os[:], O[:], rec_p[:])
                    nc.sync.dma_start(o_d.ap()[b, qp * 128 : (qp + 1) * 128, :], Os[:])

                if q < nqt:
                    # PE transposes of unnormalized expS -> PTb (bf16)
                    PTb = ptp.tile([128, kt, 128], BF16)
                    for t in range(kt):
                        ptile = ps_t.tile([128, 4, 128], F32, tag="pst", name=f"ptt{t}")
                        nc.tensor.transpose(
                            ptile[:, t % 4, :].bitcast(F32R),
                            expS[:, t * 128 : (t + 1) * 128],
                            identr[:],
                        )
                        eng = nc.vector if t % 2 == 0 else nc.scalar
                        if eng is nc.vector:
                            nc.vector.tensor_copy(
                                PTb[:, t, :], ptile[:, t % 4, :].bitcast(F32R)
                            )
                        else:
                            nc.scalar.copy(
                                PTb[:, t, :], ptile[:, t % 4, :].bitcast(F32R)
                            )
                    # normalized P write-out (off the PE path)
                    for kb in range(nkb):
                        nc.vector.tensor_scalar_mul(
                            expS[:, kb * 512 : (kb + 1) * 512],
                            expS[:, kb * 512 : (kb + 1) * 512],
                            rec[:],
                        )
                    nc.sync.dma_start(
                        p_d.ap()[b, q * 128 : (q + 1) * 128, :],
                        expS[:].bitcast(F32),
                    )
                    this = (PTb, rec)

                carry = (*this, q) if this is not None else None

    nc.compile()
    return nc


def kernel(encoder_outputs: np.ndarray, decoder_hidden: np.ndarray):
    encoder_outputs = np.ascontiguousarray(encoder_outputs, dtype=np.float32)
    decoder_hidden = np.ascontiguousarray(decoder_hidden, dtype=np.float32)
    B, Tk, H = encoder_outputs.shape
    Tq = decoder_hidden.shape[1]
    assert B % N_CORES == 0
    B_pc = B // N_CORES

    nc = build_attention(B_pc, Tq, Tk, H)
    in_maps = [
        {
            "encoder_outputs": encoder_outputs[i * B_pc : (i + 1) * B_pc],
            "decoder_hidden": decoder_hidden[i * B_pc : (i + 1) * B_pc],
        }
        for i in range(N_CORES)
    ]
    res = run_bass_kernel_spmd(nc, in_maps, core_ids=list(range(N_CORES)))
    ao = np.concatenate(
        [res.results[i]["attention_output"] for i in range(N_CORES)], axis=0
    )
    ad = np.concatenate(
        [res.results[i]["attention_distribution"] for i in range(N_CORES)], axis=0
    )
    return ao, ad


# revision 22
# speedup vs baseline: 30.4722x; 18.9462x over previous
"""Bahdanau cross-attention kernel for Trainium2 (8 NeuronCores, SPMD).

reference:
    S = dec @ enc^T            [B, Tq, Tk]
    P = softmax(S, axis=-1)    (output 2)
    O = P @ enc                [B, Tq, H]  (output 1)

Sharding: data-parallel over batch — B=16 split as 2 batches per core.
No collectives needed; each core computes its batches independently.

Per-core algorithm (per batch):
  - load enc natural, round to f32r (matmul2 rhs), and build encT
    (h on partitions) via PE transposes (matmul1 rhs).
  - per q-tile of 128 rows:
      decT via PE transposes (matmul1 stationary);
      matmul1 in 4 k-blocks of 512 accumulated over 8 h-chunks (f32r,
      full-rate); per-block rowmax + exp(S - m_b) with accumulated
      row-sums (frees the PSUM bank right away, flash-style);
      block rescale factors fix_b = e^{m_b - m}/sum folded into a single
      per-block in-place normalize -> P (f32r);
      P -> DRAM; PE-transpose P -> PT; matmul2 (PT stationary, enc_r
      moving, f32r) -> O -> DRAM.
"""

from contextlib import ExitStack

import numpy as np

import concourse.bass as bass
import concourse.tile as tile
import concourse.mybir as mybir
from concourse import bacc
from concourse.bass_utils import run_bass_kernel_spmd
from concourse.masks import make_identity

F32 = mybir.dt.float32
F32R = mybir.dt.float32r
BF16 = mybir.dt.bfloat16
AF = mybir.ActivationFunctionType
AX = mybir.AxisListType
ALU = mybir.AluOpType

N_CORES = 8


def build_attention(B_pc: int, Tq: int, Tk: int, H: int):
    assert Tq % 128 == 0 and Tk % 512 == 0 and H % 128 == 0
    kt = Tk // 128  # k tiles (PT / mm2 contraction chunks)
    hc = H // 128  # h chunks (mm1 contraction chunks)
    nkb = Tk // 512  # S blocks (512-wide)
    nqt = Tq // 128  # q tiles
    hb = min(H, 512)  # mm2 moving-block width
    nhb = H // hb

    nc = bacc.Bacc("TRN2", target_bir_lowering=False, debug=False, num_devices=N_CORES)
    enc_d = nc.dram_tensor("encoder_outputs", [B_pc, Tk, H], F32, kind="ExternalInput")
    dec_d = nc.dram_tensor("decoder_hidden", [B_pc, Tq, H], F32, kind="ExternalInput")
    o_d = nc.dram_tensor("attention_output", [B_pc, Tq, H], F32, kind="ExternalOutput")
    p_d = nc.dram_tensor(
        "attention_distribution", [B_pc, Tq, Tk], F32, kind="ExternalOutput"
    )

    with tile.TileContext(nc) as tc, ExitStack() as ctx:
        singles = ctx.enter_context(tc.tile_pool(name="singles", bufs=1))
        encp = ctx.enter_context(tc.tile_pool(name="encp", bufs=1))
        staging = ctx.enter_context(tc.tile_pool(name="staging", bufs=3))
        decp = ctx.enter_context(tc.tile_pool(name="decp", bufs=3))
        expp = ctx.enter_context(tc.tile_pool(name="expp", bufs=2))
        popb = ctx.enter_context(tc.tile_pool(name="popb", bufs=2))
        ptp = ctx.enter_context(tc.tile_pool(name="ptp", bufs=2))
        op = ctx.enter_context(tc.tile_pool(name="op", bufs=2))
        stats = ctx.enter_context(tc.tile_pool(name="stats", bufs=4))
        ps_s = ctx.enter_context(tc.tile_pool(name="ps_s", bufs=4, space="PSUM"))
        ps_t = ctx.enter_context(tc.tile_pool(name="ps_t", bufs=2, space="PSUM"))
        ps_o = ctx.enter_context(tc.tile_pool(name="ps_o", bufs=1, space="PSUM"))

        ident = singles.tile([128, 128], F32)
        make_identity(nc, ident[:])
        identb = singles.tile([128, 128], BF16)
        make_identity(nc, identb[:])
        tc.strict_bb_all_engine_barrier()

        for b in range(B_pc):
            # ---- per-batch enc setup: enc_b (k-major, bf16, mm2 rhs) +
            #      encT (h-major, f32r, mm1 rhs)
            encb = encp.tile([128, kt, H], BF16, tag="encb")
            encT = encp.tile([128, hc, Tk], F32R, tag="encT")
            for t in range(kt):
                st = staging.tile([128, H], F32)
                nc.sync.dma_start(st[:], enc_d.ap()[b, t * 128 : (t + 1) * 128, :])
                nc.vector.tensor_copy(encb[:, t, :], st[:])  # cast f32 -> bf16
                for h in range(hc):
                    ptile = ps_t.tile([128, 4, 128], F32, tag="pst")
                    nc.tensor.transpose(
                        ptile[:, h % 4, :], st[:, h * 128 : (h + 1) * 128], ident[:]
                    )
                    if h % 2 == 0:
                        nc.scalar.copy(
                            encT[:, h, t * 128 : (t + 1) * 128], ptile[:, h % 4, :]
                        )
                    else:
                        nc.vector.tensor_copy(
                            encT[:, h, t * 128 : (t + 1) * 128], ptile[:, h % 4, :]
                        )

            # ---- q-tile loop, software-pipelined:
            # iteration q emits [dec-prep](q+1), [mm1/softmax/PTb](q),
            # then [mm2/out](q-1)
            def dec_prep(q):
                dn = staging.tile([128, H], F32, tag="dec_nat", name=f"dn{q}")
                nc.sync.dma_start(dn[:], dec_d.ap()[b, q * 128 : (q + 1) * 128, :])
                decT = decp.tile([128, hc, 128], F32R, tag="decT", name=f"decT{q}")
                for h in range(hc):
                    ptile = ps_t.tile([128, 4, 128], F32, tag="pst", name=f"pt{q}_{h}")
                    nc.tensor.transpose(
                        ptile[:, h % 4, :], dn[:, h * 128 : (h + 1) * 128], ident[:]
                    )
                    nc.vector.tensor_copy(decT[:, h, :], ptile[:, h % 4, :])
                return decT

            decTs = {0: dec_prep(0)}
            carry = None
            for q in range(nqt + 1):
                if q + 1 < nqt:
                    decTs[q + 1] = dec_prep(q + 1)
                this = None
                if q < nqt:
                    decT = decTs.pop(q)
                    expS = expp.tile([128, Tk], BF16)
                    rowmaxneg = stats.tile([128, nkb], F32, tag="rmn")
                    sums = stats.tile([128, nkb], F32, tag="sums")
                    Sblk = []
                    # kb-outer: each S bank gets its 8 accumulating matmuls
                    # back-to-back (no PSUM bank cycling between matmuls)
                    for kb in range(nkb):
                        S = ps_s.tile([128, 512], F32, tag="S", name=f"S{kb}")
                        for h in range(hc):
                            nc.tensor.matmul(
                                S[:],
                                decT[:, h, :],
                                encT[:, h, kb * 512 : (kb + 1) * 512],
                                start=(h == 0),
                                stop=(h == hc - 1),
                            )
                        nc.vector.reduce_max(
                            out=rowmaxneg[:, kb : kb + 1],
                            in_=S[:],
                            axis=AX.X,
                            negate=True,
                        )
                        Sblk.append(S)
                    # global row max (negated) -> single exp bias; matmul2
                    # consumes UNNORMALIZED expS (bf16) so nothing downstream
                    # of exp gates the PE; 1/rowsum folds into the O
                    # copy-back (per-partition scale) and the P write-out.
                    mneg = stats.tile([128, 1], F32, tag="mneg")
                    nc.vector.tensor_reduce(
                        out=mneg[:], in_=rowmaxneg[:], axis=AX.X, op=ALU.min
                    )
                    for kb in range(nkb):
                        nc.scalar.activation(
                            expS[:, kb * 512 : (kb + 1) * 512],
                            Sblk[kb][:],
                            AF.Exp,
                            bias=mneg[:],
                            scale=1.0,
                            accum_out=sums[:, kb : kb + 1],
                        )
                    Sblk.clear()
                    ssum = stats.tile([128, 1], F32, tag="ssum")
                    nc.vector.reduce_sum(out=ssum[:], in_=sums[:], axis=AX.X)
                    rec = stats.tile([128, 1], F32, tag="rec")
                    nc.vector.reciprocal(rec[:], ssum[:])

                if carry is not None:
                    PTb_p, rec_p, qp = carry
                    O = ps_o.tile([128, H], F32)
                    # t-outer so both h-halves share the stationary PTb[t]
                    for t in range(kt):
                        for nb in range(nhb):
                            nc.tensor.matmul(
                                O[:, nb * hb : (nb + 1) * hb],
                                PTb_p[:, t, :],
                                encb[:, t, nb * hb : (nb + 1) * hb],
                                start=(t == 0),
                                stop=(t == kt - 1),
                            )
                    Os = op.tile([128, H], F32)
                    # copy-back fused with the 1/rowsum normalization
                    nc.scalar.mul(Os[:], O[:], rec_p[:])
                    nc.sync.dma_start(o_d.ap()[b, qp * 128 : (qp + 1) * 128, :], Os[:])

                if q < nqt:
                    # PE transposes of unnormalized expS (bf16) -> PTb (bf16)
                    PTb = ptp.tile([128, kt, 128], BF16)
                    for t in range(kt):
                        ptile = ps_t.tile([128, 4, 128], BF16, tag="pst", name=f"ptt{t}")
                        nc.tensor.transpose(
                            ptile[:, t % 4, :],
                            expS[:, t * 128 : (t + 1) * 128],
                            identb[:],
                        )
                        if t % 2 == 0:
                            nc.vector.tensor_copy(PTb[:, t, :], ptile[:, t % 4, :])
                        else:
                            nc.scalar.copy(PTb[:, t, :], ptile[:, t % 4, :])
                    # normalized P write-out in f32 (off the PE path)
                    Pout = popb.tile([128, Tk], F32)
                    for kb in range(nkb):
                        nc.vector.tensor_scalar_mul(
                            Pout[:, kb * 512 : (kb + 1) * 512],
                            expS[:, kb * 512 : (kb + 1) * 512],
                            rec[:],
                        )
                    nc.sync.dma_start(
                        p_d.ap()[b, q * 128 : (q + 1) * 128, :], Pout[:]
                    )
                    this = (PTb, rec)

                carry = (*this, q) if this is not None else None

    nc.compile()
    return nc


def kernel(encoder_outputs: np.ndarray, decoder_hidden: np.ndarray):
    encoder_outputs = np.ascontiguousarray(encoder_outputs, dtype=np.float32)
    decoder_hidden = np.ascontiguousarray(decoder_hidden, dtype=np.float32)
    B, Tk, H = encoder_outputs.shape
    Tq = decoder_hidden.shape[1]
    assert B % N_CORES == 0
    B_pc = B // N_CORES

    nc = build_attention(B_pc, Tq, Tk, H)
    in_maps = [
        {
            "encoder_outputs": encoder_outputs[i * B_pc : (i + 1) * B_pc],
            "decoder_hidden": decoder_hidden[i * B_pc : (i + 1) * B_pc],
        }
        for i in range(N_CORES)
    ]
    res = run_bass_kernel_spmd(nc, in_maps, core_ids=list(range(N_CORES)))
    ao = np.concatenate(
        [res.results[i]["attention_output"] for i in range(N_CORES)], axis=0
    )
    ad = np.concatenate(
        [res.results[i]["attention_distribution"] for i in range(N_CORES)], axis=0
    )
    return ao, ad
